# revision 1
# baseline (speedup 1.0000x reference)
"""Trainium2 Bass kernel for nn_CkConv1D (continuous-kernel causal conv).

Math: the reference builds a T x T Toeplitz kernel K[o,c,i,j] =
sum_h w2[h]*sin(A_h*(j-i) + off[o,c,h]) + b2  (A_h = w1[h,0]/T), masks it
causally (j<=i) and contracts with x.  Using sin(X+Y) = sinX cosY + cosX sinY
with X = A_h*j, Y = off - A_h*i, the masked contraction factorizes into
causal prefix sums over j of sin(A_h j)x[j,c] / cos(A_h j)x[j,c], computed
with one upper-triangular matmul per 128-row block plus block-level partial
sums.  Work is sharded over 8 NeuronCores: core m produces output rows
[128m, 128m+128).  The program is identical on every core (SPMD); per-core
behavior comes only from per-core input data (its x window, a causally
masked copy of x, and its row-index vector).

Partition layout: p = c*32 + h (C_in=4 channels x H=32 hidden = 128).
"""

import sys
from pathlib import Path

import numpy as np

for _p in ("/opt/trn_rl_repo",):
    if _p not in sys.path and Path(_p).exists():
        sys.path.insert(0, _p)

import concourse.bass as bass
import concourse.bacc as bacc
import concourse.tile as tile
from concourse import mybir
from concourse.bass_utils import run_bass_kernel_spmd

F32 = mybir.dt.float32
PI2 = float(np.pi / 2)
T, C, O, H, P, M = 1024, 4, 2, 32, 128, 8

# column offsets inside the packed "rows" [1, 2304] input
R_JJ = 0          # arange(128)
R_ONES128 = 128   # ones
R_CVEC = 256      # repeat(arange(4), 32)
R_W10x4 = 384     # tile(w1[:,0], 4)
R_W11x4 = 512     # tile(w1[:,1], 4)
R_W12x4 = 640     # tile(w1[:,2], 4)
R_B1x4 = 768      # tile(b1, 4)
R_IROW = 896      # per-core arange(128m, 128m+128)
R_ONES256 = 1024  # ones
R_OSEL = 1280     # [0]*128 + [1]*128
R_BROW = 1536     # repeat(arange(8)*128, 32)
R_W10x8 = 1792    # tile(w1[:,0], 8)
R_I2ROW = 2048    # per-core i_row twice
N_ROWS = 2304

_nc_cache = {}


def _build_nc():
    nc = bacc.Bacc()
    rows = nc.dram_tensor("rows", [1, N_ROWS], F32, kind="ExternalInput")
    ut = nc.dram_tensor("ut", [P, P], F32, kind="ExternalInput")
    xm = nc.dram_tensor("xm", [P, M, C], F32, kind="ExternalInput")
    xwin = nc.dram_tensor("xwin", [P, C], F32, kind="ExternalInput")
    w2col = nc.dram_tensor("w2col", [P, 1], F32, kind="ExternalInput")
    b2col4 = nc.dram_tensor("b2col4", [C, 1], F32, kind="ExternalInput")
    y = nc.dram_tensor("y", [1, O, P], F32, kind="ExternalOutput")

    Sin = mybir.ActivationFunctionType.Sin
    Add = mybir.AluOpType.add
    Mult = mybir.AluOpType.mult

    with tile.TileContext(nc) as tc:
        with (
            tc.tile_pool(name="sb", bufs=1) as sb,
            tc.tile_pool(name="ps", bufs=1, space="PSUM") as ps,
            tc.tile_pool(name="dr", bufs=1, space="DRAM") as dr,
        ):
            rows_sb = sb.tile([1, N_ROWS], F32)
            ut_sb = sb.tile([P, P], F32)
            xm_sb = sb.tile([P, M, C], F32)
            xwin_sb = sb.tile([P, C], F32)
            w2col_sb = sb.tile([P, 1], F32)
            b2col4_sb = sb.tile([C, 1], F32)
            nc.sync.dma_start(out=rows_sb[:], in_=rows[:])
            nc.sync.dma_start(out=ut_sb[:], in_=ut[:])
            nc.sync.dma_start(out=xm_sb[:], in_=xm[:])
            nc.sync.dma_start(out=xwin_sb[:], in_=xwin[:])
            nc.sync.dma_start(out=w2col_sb[:], in_=w2col[:])
            nc.sync.dma_start(out=b2col4_sb[:], in_=b2col4[:])

            def row(off, n):
                return rows_sb[:, off:off + n]

            # ---- tiny weight prep (single-partition DVE ops) ----
            negA4 = sb.tile([1, P], F32)
            A32 = sb.tile([1, H], F32)
            A8 = sb.tile([1, M * H], F32)
            bA = sb.tile([1, M * H], F32)
            off0 = sb.tile([1, P], F32)
            nc.vector.tensor_scalar_mul(negA4[:], row(R_W10x4, P), -1.0 / T)
            nc.vector.tensor_scalar_mul(A32[:], row(R_W10x4, H), 1.0 / T)
            nc.vector.tensor_scalar_mul(A8[:], row(R_W10x8, M * H), 1.0 / T)
            nc.vector.tensor_mul(bA[:], A8[:], row(R_BROW, M * H))
            nc.vector.tensor_mul(off0[:], row(R_CVEC, P), row(R_W11x4, P))
            nc.vector.tensor_add(off0[:], off0[:], row(R_B1x4, P))

            # ---- phase grids via K=1 outer-product matmuls ----
            # argJW bank: argJ[jj, (b,h)] = A_h*(128b + jj)  |  argW[jj, h]
            argJW = ps.tile([P, M * H + H], F32)
            argJ = argJW[:, 0:M * H].rearrange("p (b h) -> p b h", b=M)
            argW = argJW[:, M * H:M * H + H]
            nc.tensor.matmul(argJ, row(R_JJ, P), A8[:], start=True, stop=False)
            nc.tensor.matmul(argJ, row(R_ONES128, P), bA[:], start=False, stop=True)
            nc.tensor.matmul(argW, row(R_IROW, P), A32[:], start=True, stop=True)
            # argQ[p, (o,ii)] = -A_p*i + off0_p + o*w1[h,2]
            argQ = ps.tile([P, O, P], F32)
            nc.tensor.matmul(argQ[:], negA4[:], row(R_I2ROW, O * P), start=True, stop=False)
            nc.tensor.matmul(argQ[:], off0[:], row(R_ONES256, O * P), start=False, stop=False)
            nc.tensor.matmul(argQ[:], row(R_W12x4, P), row(R_OSEL, O * P), start=False, stop=True)

            # ---- sines (ScalarE LUT); cos(x) = sin(x + pi/2) ----
            pi2_col = sb.tile([P, 1], F32)
            nc.vector.memset(pi2_col[:], PI2)
            # dummy sin with no upstream deps: forces the ACT Sin table
            # load to happen at t=0 instead of serializing behind the args
            warm = sb.tile([P, 1], F32)
            nc.scalar.activation(warm[:], pi2_col[:], Sin)
            TT = sb.tile([P, 2, M, H], F32)   # [jj, sin|cos, b, h]
            nc.scalar.activation(TT[:, 0], argJ, Sin)
            nc.scalar.activation(TT[:, 1], argJ, Sin, bias=pi2_col[:])
            TW = sb.tile([P, 2, H], F32)      # [jj, sin|cos, h] own window
            nc.scalar.activation(TW[:, 0], argW, Sin)
            nc.scalar.activation(TW[:, 1], argW, Sin, bias=pi2_col[:])
            # query-side args can exceed pi; wrap into [-pi, pi] (one period
            # is enough: |argQ| + pi/2 < 3*pi for this problem's weights)
            wrS = sb.tile([P, O, P], F32)
            wrC = sb.tile([P, O, P], F32)
            nc.vector.add_range_wrap(wrS[:], argQ[:], 0.0, float(np.pi), float(2 * np.pi))
            nc.vector.add_range_wrap(wrC[:], argQ[:], PI2, float(np.pi), float(2 * np.pi))
            QT = sb.tile([P, 2, O, P], F32)   # [p, sin|cos, o, ii] query side
            nc.scalar.activation(QT[:, 0], wrS[:], Sin)
            nc.scalar.activation(QT[:, 1], wrC[:], Sin)

            # ---- window products R[jj, (c,h)] = trig[jj,h] * xwin[jj,c] ----
            R_s = sb.tile([P, C, H], F32)
            R_c = sb.tile([P, C, H], F32)
            tw_s = TW[:, 0].unsqueeze(1).broadcast_to([P, C, H])
            tw_c = TW[:, 1].unsqueeze(1).broadcast_to([P, C, H])
            xw_b = xwin_sb[:].unsqueeze(2).broadcast_to([P, C, H])
            nc.vector.tensor_mul(R_s[:], tw_s, xw_b)
            nc.vector.tensor_mul(R_c[:], tw_c, xw_b)

            # ---- contractions on PE ----
            # part1[c, (s,h)] = sum_b xm_b^T @ [TT_s | TT_c]_b   (j < 128m part)
            pc1 = ps.tile([C, 2, H], F32)
            pcx = ps.tile([C, 1], F32)
            for b in range(M):
                nc.tensor.matmul(pc1[:], xm_sb[:, b], TT[:, :, b, :],
                                 start=(b == 0), stop=(b == M - 1))
            for b in range(M):
                nc.tensor.matmul(pcx[:], xm_sb[:, b], ut_sb[:, P - 1:P],
                                 start=(b == 0), stop=(b == M - 1))
            # windowed prefix sums: pw*[p, ii] = sum_{jj<=ii} R[jj, p]
            pwS = ps.tile([P, P], F32)
            pwC = ps.tile([P, P], F32)
            pwxy = ps.tile([C, P + O * P], F32)
            pwx = pwxy[:, 0:P]
            yterm = pwxy[0:1, P:P + O * P].rearrange("a (o i) -> a o i", o=O)
            nc.tensor.matmul(pwS[:], R_s[:], ut_sb[:], start=True, stop=True)
            nc.tensor.matmul(pwC[:], R_c[:], ut_sb[:], start=True, stop=True)
            nc.tensor.matmul(pwx, xwin_sb[:], ut_sb[:], start=True, stop=True)

            # ---- reshape part1 [c, s, h] -> per-partition cols [p=(c,h), s] ----
            pc1_sb = sb.tile([C, 2, H], F32)
            nc.vector.tensor_copy(pc1_sb[:], pc1[:])
            col_s_t = sb.tile([P, 1], F32)
            col_c_t = sb.tile([P, 1], F32)
            src = pc1_sb[:]
            # src iterates (c, h), dst fills partitions p = c*32+h in order
            nc.sync.dma_start(
                out=col_s_t[:],
                in_=bass.AP(tensor=src.tensor, offset=src.offset,
                            ap=[[2 * H, C], [1, H]]))
            nc.scalar.dma_start(
                out=col_c_t[:],
                in_=bass.AP(tensor=src.tensor, offset=src.offset + H,
                            ap=[[2 * H, C], [1, H]]))
            col_s = col_s_t[:]
            col_c = col_c_t[:]

            pcx_sb = sb.tile([C, 1], F32)
            nc.vector.tensor_copy(pcx_sb[:], pcx[:])


            # ---- combine:  G[p,(o,ii)] = QC*(pwS+col_s) + QS*(pwC+col_c) ----
            G = sb.tile([P, O, P], F32)
            G2 = sb.tile([P, O, P], F32)
            pwS_b = pwS[:].unsqueeze(1).broadcast_to([P, O, P])
            pwC_b = pwC[:].unsqueeze(1).broadcast_to([P, O, P])
            nc.vector.scalar_tensor_tensor(G[:], pwS_b, col_s, QT[:, 1], Add, Mult)
            nc.vector.scalar_tensor_tensor(G2[:], pwC_b, col_c, QT[:, 0], Add, Mult)
            nc.vector.tensor_add(G[:], G[:], G2[:])

            # b2 term: t4x2[c, (o,ii)] = pwx + pcx, replicated over o
            t4a = sb.tile([C, P], F32)
            t4x2 = sb.tile([C, O, P], F32)
            nc.vector.tensor_scalar_add(t4a[:], pwx, pcx_sb[:])
            nc.vector.tensor_copy(t4x2[:], t4a[:].unsqueeze(1).broadcast_to([C, O, P]))

            # ---- final contraction over p and c ----
            nc.tensor.matmul(yterm, w2col_sb[:], G[:], start=True, stop=False)
            nc.tensor.matmul(yterm, b2col4_sb[:], t4x2[:], start=False, stop=True)
            ysb = sb.tile([1, O, P], F32)
            nc.vector.tensor_copy(ysb[:], yterm)
            nc.sync.dma_start(out=y[:], in_=ysb[:])
    nc.finalize()
    return nc


def _host_inputs(x, w1, b1, w2, b2):
    """Per-core input maps.  Host does only layout/replication/masking."""
    x = np.ascontiguousarray(x, np.float32)
    w1 = np.asarray(w1, np.float32)
    b1 = np.asarray(b1, np.float32)
    w2 = np.asarray(w2, np.float32)
    b2 = np.asarray(b2, np.float32)

    base = np.zeros(N_ROWS, np.float32)
    base[R_JJ:R_JJ + P] = np.arange(P)
    base[R_ONES128:R_ONES128 + P] = 1.0
    base[R_CVEC:R_CVEC + P] = np.repeat(np.arange(C), H)
    base[R_W10x4:R_W10x4 + P] = np.tile(w1[:, 0], C)
    base[R_W11x4:R_W11x4 + P] = np.tile(w1[:, 1], C)
    base[R_W12x4:R_W12x4 + P] = np.tile(w1[:, 2], C)
    base[R_B1x4:R_B1x4 + P] = np.tile(b1, C)
    base[R_ONES256:R_ONES256 + O * P] = 1.0
    base[R_OSEL + P:R_OSEL + O * P] = 1.0
    base[R_BROW:R_BROW + M * H] = np.repeat(np.arange(M) * P, H)
    base[R_W10x8:R_W10x8 + M * H] = np.tile(w1[:, 0], M)

    ut = np.triu(np.ones((P, P), np.float32))
    w2c = np.tile(w2[0], C)[:, None].astype(np.float32)
    b2c = np.full((C, 1), b2[0], np.float32)
    xr = x.reshape(M, P, C)

    in_maps = []
    for m in range(M):
        rows = base.copy()
        i_vals = (np.arange(P) + P * m).astype(np.float32)
        rows[R_IROW:R_IROW + P] = i_vals
        rows[R_I2ROW:R_I2ROW + P] = i_vals
        rows[R_I2ROW + P:R_I2ROW + O * P] = i_vals
        xmask = x.copy()
        xmask[P * m:] = 0.0
        xm = np.ascontiguousarray(xmask.reshape(M, P, C).transpose(1, 0, 2))
        in_maps.append({
            "rows": rows[None, :],
            "ut": ut,
            "xm": xm,
            "xwin": xr[m],
            "w2col": w2c,
            "b2col4": b2c,
        })
    return in_maps


def kernel(x, t, w1, b1, w2, b2, out_channels):
    if "nc" not in _nc_cache:
        _nc_cache["nc"] = _build_nc()
    nc = _nc_cache["nc"]
    in_maps = _host_inputs(x, w1, b1, w2, b2)
    res = run_bass_kernel_spmd(nc, in_maps, core_ids=list(range(M)))
    y = np.empty((T, O), np.float32)
    for m in range(M):
        ym = np.asarray(res.results[m]["y"]).reshape(O, P)
        y[P * m:P * (m + 1), :] = ym.T
    return y



# revision 4
# speedup vs baseline: 1.3688x; 1.3688x over previous
"""Trainium2 Bass kernel for nn_CkConv1D (continuous-kernel causal conv).

Math: the reference builds a T x T Toeplitz kernel K[o,c,i,j] =
sum_h w2[h]*sin(A_h*(j-i) + off[o,c,h]) + b2  (A_h = w1[h,0]/T), masks it
causally (j<=i) and contracts with x [T, C].  Since K depends only on
(j - i), everything is phrased in LOCAL window coordinates (ii = i mod 128,
jj = j mod 128): with theta[p,(o,ii)] = off0_p + o*w12_p - A_p*ii,

  y[i,o] = sum_p w2_p * [cos(theta)*(pwS + histS)_p + sin(theta)*(pwC + histC)_p]
           + b2 * (pwx + pcx)[ii]

where pwS/pwC are causal window prefix sums of sin(A jj)*x / cos(A jj)*x
(one upper-triangular matmul each), and the history term comes from
per-block partial sums P[(b,c),(t,h)] = xblk^T @ [sin|cos](A jj) rotated by
block phases 128*A*(b-m) and summed over blocks b<m (tiny masked matmul).

Sharded over 8 NeuronCores: core m computes output rows [128m, 128m+128).
SPMD: identical program, per-core behavior comes only from input data
(x window slice, block-phase grids, block mask).  Host prep is limited to
layout/replication and affine iota*weight phase grids (pre-wrapped into
[-pi, pi) because the ACT Sin LUT is only accurate there); all sines,
x contractions and T^2-scale work happen on device.

Partition layout: p = c*32 + h (C_in=4 channels x H=32 hidden = 128).
"""

import sys
from pathlib import Path

import numpy as np

for _p in ("/opt/trn_rl_repo",):
    if _p not in sys.path and Path(_p).exists():
        sys.path.insert(0, _p)

import concourse.bass as bass
import concourse.bacc as bacc
import concourse.tile as tile
from concourse import mybir
from concourse.bass_utils import run_bass_kernel_spmd

F32 = mybir.dt.float32
F32R = mybir.dt.float32r
BF16 = mybir.dt.bfloat16
F16 = mybir.dt.float16
PI = float(np.pi)
PI2 = float(np.pi / 2)
T, C, O, H, P, M = 1024, 4, 2, 32, 128, 8

# D2 (2-byte tensor) column offsets.  bf16 columns hold bf16 data; "grid"
# columns hold raw fp16 bits (bitcast to F16 on device before ACT).
D_UT = 0          # [128, 128] bf16 upper-tri (jj <= ii)
D_XWIN = 128      # [128, 4]   bf16 own x window
D_XBLK = 132      # [128, 32]  bf16 x blocked [jj, (b,c)]
D_QTSG = 164      # [128, 256] fp16 wrap(theta)        -> sin side
D_QTCG = 420      # [128, 256] fp16 wrap(theta + pi/2) -> cos side
D_ARGL = 676      # [128, 32]  fp16 A_h * jj (local window phases)
D_ARGBS = 708     # [32, 32]   fp16 wrap(128*A*(b-m))        (rows 0:32)
D_ARGBC = 740     # [32, 32]   fp16 wrap(128*A*(b-m) + pi/2) (rows 0:32)
D_N = 772

# DW (float32r tensor) columns
W_W2 = 0          # [128, 1] w2 tiled over c
W_B2 = 1          # [4, 1]   b2 (rows 0:4)
W_E4M = 2         # [32, 4]  E4M[(b,c), c'] = (c==c') & (b<m)  (rows 0:32)
W_N = 6

ACT_F32R = True   # Scalar ACT writes float32r directly (else cast on DVE)

_nc_cache = {}


def _build_nc():
    nc = bacc.Bacc()
    d2 = nc.dram_tensor("d2", [P, D_N], BF16, kind="ExternalInput")
    dw = nc.dram_tensor("dw", [P, W_N], F32R, kind="ExternalInput")
    y = nc.dram_tensor("y", [1, O, P], F32, kind="ExternalOutput")

    Sin = mybir.ActivationFunctionType.Sin
    QT_DT = F32R if ACT_F32R else F32

    with tile.TileContext(nc) as tc:
        with (
            tc.tile_pool(name="sb", bufs=1) as sb,
            tc.tile_pool(name="ps", bufs=1, space="PSUM") as ps,
        ):
            d2_sb = sb.tile([P, D_N], BF16)
            dw_sb = sb.tile([P, W_N], F32R)
            nc.sync.dma_start(out=d2_sb[:], in_=d2[:])
            nc.scalar.dma_start(out=dw_sb[:], in_=dw[:])

            def grid(off, n, rows=P):
                return d2_sb[0:rows, off:off + n].bitcast(F16)

            # dummy sin with no upstream deps: forces the ACT Sin table
            # load to happen at t=0 instead of serializing behind the DMAs
            pi2c = sb.tile([P, 1], F32)
            nc.vector.memset(pi2c[:], PI2)
            warm = sb.tile([P, 1], F32)
            nc.scalar.activation(warm[:], pi2c[:], Sin)

            # ---- trig tables (ScalarE LUT; cos(x) = sin(x + pi/2)) ----
            TLp = sb.tile([P, 2 * H + 1], BF16)   # [jj, sin|cos|ones]
            nc.scalar.activation(TLp[:, 0:H], grid(D_ARGL, H), Sin)
            nc.scalar.activation(TLp[:, H:2 * H], grid(D_ARGL, H), Sin,
                                 bias=pi2c[:])
            nc.vector.memset(TLp[:, 2 * H:2 * H + 1], 1.0)
            phS = sb.tile([H, H], F32)            # block phases [(b,c), h]
            phC = sb.tile([H, H], F32)
            nc.scalar.activation(phS[:], grid(D_ARGBS, H, rows=H), Sin)
            nc.scalar.activation(phC[:], grid(D_ARGBC, H, rows=H), Sin)
            QTs = sb.tile([P, O, P], QT_DT)       # query side [p, o, ii]
            QTc = sb.tile([P, O, P], QT_DT)
            nc.scalar.activation(
                QTs[:].rearrange("p o i -> p (o i)"), grid(D_QTSG, O * P), Sin)
            nc.scalar.activation(
                QTc[:].rearrange("p o i -> p (o i)"), grid(D_QTCG, O * P), Sin)

            # ---- window products R[jj, (c,h)] = trig[jj,h] * xwin[jj,c] ----
            R_s = sb.tile([P, C, H], BF16)
            R_c = sb.tile([P, C, H], BF16)
            tl_s = TLp[:, 0:H].unsqueeze(1).broadcast_to([P, C, H])
            tl_c = TLp[:, H:2 * H].unsqueeze(1).broadcast_to([P, C, H])
            xw_b = d2_sb[:, D_XWIN:D_XWIN + C].unsqueeze(2).broadcast_to([P, C, H])
            nc.vector.tensor_mul(R_s[:], tl_s, xw_b)
            nc.vector.tensor_mul(R_c[:], tl_c, xw_b)

            # ---- PE: history partials, then window prefix sums ----
            p_ps = ps.tile([H, 2 * H + 1], F32)   # P[(b,c), (sin|cos,h)|ones]
            nc.tensor.matmul(p_ps[:], d2_sb[:, D_XBLK:D_XBLK + H], TLp[:],
                             start=True, stop=True)
            ut = d2_sb[:, D_UT:D_UT + P]
            pwS = ps.tile([P, P], F32)
            pwC = ps.tile([P, P], F32)
            pwx = ps.tile([C, P], F32)
            nc.tensor.matmul(pwS[:], R_s[:], ut, start=True, stop=True)
            nc.tensor.matmul(pwC[:], R_c[:], ut, start=True, stop=True)
            nc.tensor.matmul(pwx[:], d2_sb[:, D_XWIN:D_XWIN + C], ut,
                             start=True, stop=True)

            # ---- history: rotate partials by block phases, mask+sum b<m ----
            Ps, Pc = p_ps[:, 0:H], p_ps[:, H:2 * H]
            t_a = sb.tile([H, H], F32)
            t_b = sb.tile([H, H], F32)
            Qsc = sb.tile([H, 2 * H + 1], F32)    # [ (b,c), (Qs|Qc|ones) ]
            nc.vector.tensor_mul(t_a[:], phC[:], Ps)
            nc.vector.tensor_mul(t_b[:], phS[:], Pc)
            nc.vector.tensor_add(Qsc[:, 0:H], t_a[:], t_b[:])
            nc.vector.tensor_mul(t_a[:], phC[:], Pc)
            nc.vector.tensor_mul(t_b[:], phS[:], Ps)
            nc.vector.tensor_sub(Qsc[:, H:2 * H], t_a[:], t_b[:])
            nc.vector.tensor_copy(Qsc[:, 2 * H:2 * H + 1], p_ps[:, 2 * H:2 * H + 1])
            hx = ps.tile([C, 2 * H + 1], F32)     # [c, (histS|histC,h)|pcx]
            nc.tensor.matmul(hx[:], dw_sb[0:H, W_E4M:W_E4M + C].bitcast(F32),
                             Qsc[:], start=True, stop=True)
            hxs = sb.tile([C, 2 * H + 1], F32)
            nc.vector.tensor_copy(hxs[:], hx[:])

            # hist [c, h] -> per-partition columns [(c,h), 1] via gather DMA
            col_s = sb.tile([P, 1], F32)
            col_c = sb.tile([P, 1], F32)
            src = hxs[:]
            nc.sync.dma_start(
                out=col_s[:],
                in_=bass.AP(tensor=src.tensor, offset=src.offset,
                            ap=[[2 * H + 1, C], [1, H]]))
            nc.scalar.dma_start(
                out=col_c[:],
                in_=bass.AP(tensor=src.tensor, offset=src.offset + H,
                            ap=[[2 * H + 1, C], [1, H]]))

            # ---- combine on DVE (no col dependency: hist goes via PE) ----
            G1 = sb.tile([P, O, P], F32R)
            G2 = sb.tile([P, O, P], F32R)
            pwS_b = pwS[:].unsqueeze(1).broadcast_to([P, O, P])
            pwC_b = pwC[:].unsqueeze(1).broadcast_to([P, O, P])
            nc.vector.tensor_mul(G1[:], pwS_b, QTc[:])
            nc.vector.tensor_mul(G2[:], pwC_b, QTs[:])
            if not ACT_F32R:
                QTs_r = sb.tile([P, O, P], F32R)
                QTc_r = sb.tile([P, O, P], F32R)
                nc.vector.tensor_copy(QTs_r[:], QTs[:])
                nc.vector.tensor_copy(QTc_r[:], QTc[:])
            else:
                QTs_r, QTc_r = QTs, QTc
            # b2 term: t4x2[c, (o,ii)] = pwx + pcx, replicated over o
            t4a = sb.tile([C, P], F32)
            t4x2 = sb.tile([C, O, P], F32)
            nc.vector.tensor_scalar_add(t4a[:], pwx[:], hxs[:, 2 * H:2 * H + 1])
            nc.vector.tensor_copy(t4x2[:], t4a[:].unsqueeze(1).broadcast_to([C, O, P]))
            # w2-scaled hist columns feed the final contraction directly
            wcol_s = sb.tile([P, 1], F32R)
            wcol_c = sb.tile([P, 1], F32R)
            nc.vector.tensor_mul(wcol_s[:], dw_sb[:, W_W2:W_W2 + 1], col_s[:])
            nc.vector.tensor_mul(wcol_c[:], dw_sb[:, W_W2:W_W2 + 1], col_c[:])

            # ---- final contraction over p (and c for the b2 term) ----
            yterm = ps.tile([1, O * P], F32)
            w2col = dw_sb[:, W_W2:W_W2 + 1]
            flat = lambda ap: ap.rearrange("p o i -> p (o i)")
            nc.tensor.matmul(yterm[:], w2col, flat(G1[:]), start=True, stop=False)
            nc.tensor.matmul(yterm[:], w2col, flat(G2[:]), start=False, stop=False)
            nc.tensor.matmul(yterm[:], wcol_s[:], flat(QTc_r[:]), start=False, stop=False)
            nc.tensor.matmul(yterm[:], wcol_c[:], flat(QTs_r[:]), start=False, stop=False)
            nc.tensor.matmul(yterm[:], dw_sb[0:C, W_B2:W_B2 + 1].bitcast(F32),
                             flat(t4x2[:]), start=False, stop=True)
            ysb = sb.tile([1, O, P], F32)
            nc.vector.tensor_copy(ysb[:].rearrange("p o i -> p (o i)"), yterm[:])
            nc.sync.dma_start(out=y[:], in_=ysb[:])
    nc.finalize()
    return nc


def _wrap(v):
    return (v + np.pi) % (2 * np.pi) - np.pi


def _host_inputs(x, w1, b1, w2, b2):
    """Per-core input maps.  Host does layout/replication/masking and
    affine iota*weight phase grids (pre-wrapped for the LUT range)."""
    bf16 = mybir.dt.np(BF16)
    x = np.ascontiguousarray(x, np.float32)
    w1 = np.asarray(w1, np.float64)
    b1 = np.asarray(b1, np.float64)
    w2 = np.asarray(w2, np.float32)
    b2 = np.asarray(b2, np.float32)

    A = w1[:, 0] / T                                    # [H]
    jj = np.arange(P, dtype=np.float64)
    argL = np.outer(jj, A)                              # [128, 32]
    cidx = np.arange(C, dtype=np.float64)
    off0 = cidx[:, None] * w1[None, :, 1] + b1[None, :] # [C, H]
    oidx = np.arange(O, dtype=np.float64)
    theta = (off0[:, :, None, None]
             + oidx[None, None, :, None] * w1[None, :, 2, None, None]
             - A[None, :, None, None] * jj[None, None, None, :])  # [C,H,O,P]
    qtsg = _wrap(theta).reshape(P, O * P)
    qtcg = _wrap(theta + np.pi / 2).reshape(P, O * P)

    ut = np.triu(np.ones((P, P), np.float32))
    xblk = x.reshape(M, P, C).transpose(1, 0, 2).reshape(P, M * C)

    d2_base = np.zeros((P, D_N), dtype=bf16)
    d2_base[:, D_UT:D_UT + P] = ut.astype(bf16)
    d2_base[:, D_XBLK:D_XBLK + H] = xblk.astype(bf16)
    d2_base[:, D_QTSG:D_QTSG + O * P] = qtsg.astype(np.float16).view(bf16)
    d2_base[:, D_QTCG:D_QTCG + O * P] = qtcg.astype(np.float16).view(bf16)
    d2_base[:, D_ARGL:D_ARGL + H] = argL.astype(np.float16).view(bf16)

    dw_base = np.zeros((P, W_N), np.float32)
    dw_base[:, W_W2] = np.tile(w2[0], C)
    dw_base[0:C, W_B2] = b2[0]

    bvals = np.repeat(np.arange(M, dtype=np.float64), C)  # [(b,c)] -> b
    cvals = np.tile(np.arange(C), M)                      # [(b,c)] -> c
    in_maps = []
    for m in range(M):
        d2m = d2_base.copy()
        d2m[:, D_XWIN:D_XWIN + C] = x[P * m:P * (m + 1)].astype(bf16)
        argB = np.outer(P * (bvals - m), A)               # [32, 32]
        d2m[0:H, D_ARGBS:D_ARGBS + H] = _wrap(argB).astype(np.float16).view(bf16)
        d2m[0:H, D_ARGBC:D_ARGBC + H] = _wrap(argB + np.pi / 2).astype(np.float16).view(bf16)
        dwm = dw_base.copy()
        e4m = (cvals[:, None] == np.arange(C)[None, :]) & (bvals[:, None] < m)
        dwm[0:H, W_E4M:W_E4M + C] = e4m.astype(np.float32)
        in_maps.append({"d2": d2m, "dw": dwm})
    return in_maps


def kernel(x, t, w1, b1, w2, b2, out_channels):
    if "nc" not in _nc_cache:
        _nc_cache["nc"] = _build_nc()
    nc = _nc_cache["nc"]
    in_maps = _host_inputs(x, w1, b1, w2, b2)
    res = run_bass_kernel_spmd(nc, in_maps, core_ids=list(range(M)))
    y = np.empty((T, O), np.float32)
    for m in range(M):
        ym = np.asarray(res.results[m]["y"]).reshape(O, P)
        y[P * m:P * (m + 1), :] = ym.T
    return y


# revision 10
# speedup vs baseline: 1.3987x; 1.0218x over previous
"""Trainium2 Bass kernel for nn_CkConv1D (continuous-kernel causal conv).

Math: the reference builds a T x T Toeplitz kernel K[o,c,i,j] =
sum_h w2[h]*sin(A_h*(j-i) + off[o,c,h]) + b2  (A_h = w1[h,0]/T), masks it
causally (j<=i) and contracts with x [T, C].  Since K depends only on
(j - i), everything is phrased in LOCAL window coordinates (ii = i mod 128,
jj = j mod 128): with theta[p,(o,ii)] = off0_p + o*w12_p - A_p*ii,

  y[i,o] = sum_p w2_p * [cos(theta)*(pwS + histS)_p + sin(theta)*(pwC + histC)_p]
           + b2 * (pwx + pcx)[ii]

where pwS/pwC are causal window prefix sums of sin(A jj)*x / cos(A jj)*x
(one upper-triangular matmul each), and the history term comes from
per-block partial sums P[(b,c),(t,h)] = xblk^T @ [sin|cos](A jj) rotated by
block phases 128*A*(b-m) and summed over blocks b<m (tiny masked matmul).

Sharded over 8 NeuronCores: core m computes output rows [128m, 128m+128).
SPMD: identical program, per-core behavior comes only from input data
(x window slice, block-phase grids, block mask).  Host prep is limited to
layout/replication and affine iota*weight phase grids (pre-wrapped into
[-pi, pi) because the ACT Sin LUT is only accurate there); all sines,
x contractions and T^2-scale work happen on device.

Partition layout: p = c*32 + h (C_in=4 channels x H=32 hidden = 128).
"""

import sys
from pathlib import Path

import numpy as np

for _p in ("/opt/trn_rl_repo",):
    if _p not in sys.path and Path(_p).exists():
        sys.path.insert(0, _p)

import concourse.bass as bass
import concourse.bacc as bacc
import concourse.tile as tile
from concourse import mybir
from concourse.bass_utils import run_bass_kernel_spmd

F32 = mybir.dt.float32
F32R = mybir.dt.float32r
BF16 = mybir.dt.bfloat16
F16 = mybir.dt.float16
PI = float(np.pi)
PI2 = float(np.pi / 2)
T, C, O, H, P, M = 1024, 4, 2, 32, 128, 8

# D2 (2-byte tensor) column offsets.  bf16 columns hold bf16 data; "grid"
# columns hold raw fp16 bits (bitcast to F16 on device before ACT).
D_UT = 0          # [128, 128] bf16 upper-tri (jj <= ii)
D_XWIN = 128      # [128, 4]   bf16 own x window
D_XBLK = 132      # [128, 32]  bf16 x blocked [jj, (b,c)]
D_QTSG = 164      # [128, 256] fp16 wrap(theta)        -> sin side
D_QTCG = 420      # [128, 256] fp16 wrap(theta + pi/2) -> cos side
D_ARGL = 676      # [128, 64]  fp16 [A_h*jj | A_h*jj + pi/2] window phases
D_ARGBS = 740     # [32, 32]   fp16 wrap(128*A*(b-m))        (rows 0:32)
D_ARGBC = 772     # [32, 32]   fp16 wrap(128*A*(b-m) + pi/2) (rows 0:32)
D_B2 = 804        # [4, 1]    bf16 b2 (rows 0:4)
D_N = 805

# DW (float32r tensor) columns
W_W2 = 0          # [128, 1] w2 tiled over c
W_B2 = 1          # [4, 1]   b2 (rows 0:4)
W_E4M = 2         # [32, 4]  E4M[(b,c), c'] = (c==c') & (b<m)  (rows 0:32)
W_N = 6

ACT_F32R = True   # Scalar ACT writes float32r directly (else cast on DVE)

_nc_cache = {}


def _build_nc():
    nc = bacc.Bacc()
    d2 = nc.dram_tensor("d2", [P, D_N], BF16, kind="ExternalInput")
    dw = nc.dram_tensor("dw", [P, W_N], F32R, kind="ExternalInput")
    y = nc.dram_tensor("y", [1, O, P], F32, kind="ExternalOutput")

    Sin = mybir.ActivationFunctionType.Sin
    QT_DT = F32R if ACT_F32R else F32

    with tile.TileContext(nc) as tc:
        with (
            tc.tile_pool(name="sb", bufs=1) as sb,
            tc.tile_pool(name="ps", bufs=1, space="PSUM") as ps,
        ):
            d2_sb = sb.tile([P, D_N], BF16)
            dw_sb = sb.tile([P, W_N], F32R)
            nc.sync.dma_start(out=d2_sb[:], in_=d2[:])
            nc.scalar.dma_start(out=dw_sb[:], in_=dw[:])

            def grid(off, n, rows=P):
                return d2_sb[0:rows, off:off + n].bitcast(F16)

            # dummy sin with no upstream deps: forces the ACT Sin table
            # load to happen at t=0 instead of serializing behind the DMAs.
            # fp16 input on purpose: the LUT table is keyed on input dtype
            # and every real ACT here reads fp16 grids.
            pi2h = sb.tile([P, 1], F16)
            nc.vector.memset(pi2h[:], PI2)
            warm = sb.tile([P, 1], F32)
            nc.scalar.activation(warm[:], pi2h[:], Sin)

            # ---- trig tables (one fused ACT per grid pair) ----
            TLp = sb.tile([P, 2 * H + 1], BF16)   # [jj, sin|cos|ones]
            nc.scalar.activation(TLp[:, 0:2 * H], grid(D_ARGL, 2 * H), Sin)
            nc.vector.memset(TLp[:, 2 * H:2 * H + 1], 1.0)
            phSC = sb.tile([H, 2 * H], F32)       # block phases [(b,c), h]
            nc.scalar.activation(phSC[:], grid(D_ARGBS, 2 * H, rows=H), Sin)
            phS, phC = phSC[:, 0:H], phSC[:, H:2 * H]
            QT = sb.tile([P, 2, O, P], QT_DT)     # query side [p, s|c, o, ii]
            nc.scalar.activation(
                QT[:].rearrange("p t o i -> p (t o i)"),
                grid(D_QTSG, 2 * O * P), Sin)
            QTs, QTc = QT[:, 0], QT[:, 1]

            # ---- window products R[jj, (c,h)] = trig[jj,h] * xwin[jj,c] ----
            R_s = sb.tile([P, C, H], BF16)
            R_c = sb.tile([P, C, H], BF16)
            tl_s = TLp[:, 0:H].unsqueeze(1).broadcast_to([P, C, H])
            tl_c = TLp[:, H:2 * H].unsqueeze(1).broadcast_to([P, C, H])
            xw_b = d2_sb[:, D_XWIN:D_XWIN + C].unsqueeze(2).broadcast_to([P, C, H])
            nc.vector.tensor_mul(R_s[:], tl_s, xw_b)
            nc.vector.tensor_mul(R_c[:], tl_c, xw_b)

            # ---- PE: history partials, then window prefix sums ----
            p_ps = ps.tile([H, 2 * H + 1], F32)   # P[(b,c), (sin|cos,h)|ones]
            nc.tensor.matmul(p_ps[:], d2_sb[:, D_XBLK:D_XBLK + H], TLp[:],
                             start=True, stop=True)
            ut = d2_sb[:, D_UT:D_UT + P]
            pwS = ps.tile([P, P], F32)
            pwC = ps.tile([P, P], F32)
            pwx = ps.tile([C, P], F32)
            nc.tensor.matmul(pwS[:], R_s[:], ut, start=True, stop=True)
            nc.tensor.matmul(pwC[:], R_c[:], ut, start=True, stop=True)
            nc.tensor.matmul(pwx[:], d2_sb[:, D_XWIN:D_XWIN + C], ut,
                             start=True, stop=True)

            # ---- history: rotate partials by block phases, mask+sum b<m ----
            Ps, Pc = p_ps[:, 0:H], p_ps[:, H:2 * H]
            t_a = sb.tile([H, H], F32)
            t_b = sb.tile([H, H], F32)
            # Qsc free layout is (h, s|c) interleaved so the hist gather
            # DMA below reads contiguous 64-element runs per channel
            Qsc = sb.tile([H, 2 * H + 1], F32)    # [ (b,c), (h x Qs|Qc)|ones ]
            Qv = Qsc[:, 0:2 * H].rearrange("p (h t) -> p h t", t=2)
            nc.vector.tensor_mul(t_a[:], phC, Ps)
            nc.vector.tensor_mul(t_b[:], phS, Pc)
            nc.vector.tensor_add(Qv[:, :, 0], t_a[:], t_b[:])
            nc.vector.tensor_mul(t_a[:], phC, Pc)
            nc.vector.tensor_mul(t_b[:], phS, Ps)
            nc.vector.tensor_sub(Qv[:, :, 1], t_a[:], t_b[:])
            nc.vector.tensor_copy(Qsc[:, 2 * H:2 * H + 1], p_ps[:, 2 * H:2 * H + 1])
            hx = ps.tile([C, 2 * H + 1], F32)     # [c, (h x histS|histC)|pcx]
            nc.tensor.matmul(hx[:], dw_sb[0:H, W_E4M:W_E4M + C].bitcast(F32),
                             Qsc[:], start=True, stop=True)
            hxs = sb.tile([C, 2 * H + 1], F32)
            nc.vector.tensor_copy(hxs[:], hx[:])

            # hist [c, (s|c,h)] -> per-partition columns [(c,h), s|c]
            # via one gather DMA (src order: c outer, h, then s|c inner)
            col2 = sb.tile([P, 2], F32)
            hsrc = hxs[:]
            nc.sync.dma_start(
                out=col2[:],
                in_=bass.AP(tensor=hsrc.tensor, offset=hsrc.offset,
                            ap=[[2 * H + 1, C], [1, 2 * H]]))
            col_s, col_c = col2[:, 0:1], col2[:, 1:2]

            # ---- combine on DVE (no col dependency: hist goes via PE) ----
            G1 = sb.tile([P, O, P], F32R)
            G2 = sb.tile([P, O, P], F32R)
            pwS_b = pwS[:].unsqueeze(1).broadcast_to([P, O, P])
            pwC_b = pwC[:].unsqueeze(1).broadcast_to([P, O, P])
            nc.vector.tensor_mul(G1[:], pwS_b, QTc)
            nc.vector.tensor_mul(G2[:], pwC_b, QTs)
            QTs_r, QTc_r = QTs, QTc
            # b2 term: t4x2[c, (o,ii)] = pwx + pcx, replicated over o
            t4x2 = sb.tile([C, O, P], BF16)
            pwx_b = pwx[:].unsqueeze(1).broadcast_to([C, O, P])
            nc.vector.tensor_scalar_add(t4x2[:], pwx_b, hxs[:, 2 * H:2 * H + 1])
            # w2-scaled hist columns feed the final contraction directly
            wcol_s = sb.tile([P, 1], F32R)
            wcol_c = sb.tile([P, 1], F32R)
            nc.vector.tensor_mul(wcol_s[:], dw_sb[:, W_W2:W_W2 + 1], col_s)
            nc.vector.tensor_mul(wcol_c[:], dw_sb[:, W_W2:W_W2 + 1], col_c)

            # ---- final contraction over p (and c for the b2 term) ----
            yterm = ps.tile([1, O * P], F32)
            w2col = dw_sb[:, W_W2:W_W2 + 1]
            flat = lambda ap: ap.rearrange("p o i -> p (o i)")
            nc.tensor.matmul(yterm[:], w2col, flat(G1[:]), start=True, stop=False)
            nc.tensor.matmul(yterm[:], w2col, flat(G2[:]), start=False, stop=False)
            nc.tensor.matmul(yterm[:], wcol_s[:], flat(QTc_r), start=False, stop=False)
            nc.tensor.matmul(yterm[:], wcol_c[:], flat(QTs_r), start=False, stop=False)
            nc.tensor.matmul(yterm[:], d2_sb[0:C, D_B2:D_B2 + 1],
                             flat(t4x2[:]), start=False, stop=True)
            ysb = sb.tile([1, O * P], F32)
            nc.vector.tensor_copy(ysb[:], yterm[:])
            nc.sync.dma_start(out=y[:].rearrange("p o i -> p (o i)"), in_=ysb[:])
    nc.finalize()
    return nc


def _wrap(v):
    return (v + np.pi) % (2 * np.pi) - np.pi


def _host_inputs(x, w1, b1, w2, b2):
    """Per-core input maps.  Host does layout/replication/masking and
    affine iota*weight phase grids (pre-wrapped for the LUT range)."""
    bf16 = mybir.dt.np(BF16)
    x = np.ascontiguousarray(x, np.float32)
    w1 = np.asarray(w1, np.float64)
    b1 = np.asarray(b1, np.float64)
    w2 = np.asarray(w2, np.float32)
    b2 = np.asarray(b2, np.float32)

    A = w1[:, 0] / T                                    # [H]
    jj = np.arange(P, dtype=np.float64)
    argL = np.outer(jj, A)                              # [128, 32]
    cidx = np.arange(C, dtype=np.float64)
    off0 = cidx[:, None] * w1[None, :, 1] + b1[None, :] # [C, H]
    oidx = np.arange(O, dtype=np.float64)
    theta = (off0[:, :, None, None]
             + oidx[None, None, :, None] * w1[None, :, 2, None, None]
             - A[None, :, None, None] * jj[None, None, None, :])  # [C,H,O,P]
    qtsg = _wrap(theta).reshape(P, O * P)
    qtcg = _wrap(theta + np.pi / 2).reshape(P, O * P)

    ut = np.triu(np.ones((P, P), np.float32))
    xblk = x.reshape(M, P, C).transpose(1, 0, 2).reshape(P, M * C)

    d2_base = np.zeros((P, D_N), dtype=bf16)
    d2_base[:, D_UT:D_UT + P] = ut.astype(bf16)
    d2_base[:, D_XBLK:D_XBLK + H] = xblk.astype(bf16)
    d2_base[:, D_QTSG:D_QTSG + O * P] = qtsg.astype(np.float16).view(bf16)
    d2_base[:, D_QTCG:D_QTCG + O * P] = qtcg.astype(np.float16).view(bf16)
    d2_base[:, D_ARGL:D_ARGL + H] = argL.astype(np.float16).view(bf16)
    d2_base[:, D_ARGL + H:D_ARGL + 2 * H] = (argL + np.pi / 2).astype(np.float16).view(bf16)
    d2_base[0:C, D_B2] = np.full(C, b2[0]).astype(bf16)

    dw_base = np.zeros((P, W_N), np.float32)
    dw_base[:, W_W2] = np.tile(w2[0], C)

    bvals = np.repeat(np.arange(M, dtype=np.float64), C)  # [(b,c)] -> b
    cvals = np.tile(np.arange(C), M)                      # [(b,c)] -> c
    in_maps = []
    for m in range(M):
        d2m = d2_base.copy()
        d2m[:, D_XWIN:D_XWIN + C] = x[P * m:P * (m + 1)].astype(bf16)
        argB = np.outer(P * (bvals - m), A)               # [32, 32]
        d2m[0:H, D_ARGBS:D_ARGBS + H] = _wrap(argB).astype(np.float16).view(bf16)
        d2m[0:H, D_ARGBC:D_ARGBC + H] = _wrap(argB + np.pi / 2).astype(np.float16).view(bf16)
        dwm = dw_base.copy()
        e4m = (cvals[:, None] == np.arange(C)[None, :]) & (bvals[:, None] < m)
        dwm[0:H, W_E4M:W_E4M + C] = e4m.astype(np.float32)
        in_maps.append({"d2": d2m, "dw": dwm})
    return in_maps


def kernel(x, t, w1, b1, w2, b2, out_channels):
    if "nc" not in _nc_cache:
        _nc_cache["nc"] = _build_nc()
    nc = _nc_cache["nc"]
    in_maps = _host_inputs(x, w1, b1, w2, b2)
    res = run_bass_kernel_spmd(nc, in_maps, core_ids=list(range(M)))
    y = np.empty((T, O), np.float32)
    for m in range(M):
        ym = np.asarray(res.results[m]["y"]).reshape(O, P)
        y[P * m:P * (m + 1), :] = ym.T
    return y


# revision 11
# speedup vs baseline: 1.5096x; 1.0793x over previous
"""Trainium2 Bass kernel for nn_CkConv1D (continuous-kernel causal conv).

Math: the reference builds a T x T Toeplitz kernel K[o,c,i,j] =
sum_h w2[h]*sin(A_h*(j-i) + off[o,c,h]) + b2  (A_h = w1[h,0]/T), masks it
causally (j<=i) and contracts with x [T, C].  Since K depends only on
(j - i), everything is phrased in LOCAL window coordinates (ii = i mod 128,
jj = j mod 128): with theta[p,(o,ii)] = off0_p + o*w12_p - A_p*ii,

  y[i,o] = sum_p w2_p * [cos(theta)*(pwS + histS)_p + sin(theta)*(pwC + histC)_p]
           + b2 * (pwx + pcx)[ii]

where pwS/pwC are causal window prefix sums of sin(A jj)*x / cos(A jj)*x
(one upper-triangular matmul each), and the history term comes from
per-block partial sums P[(b,c),(t,h)] = xblk^T @ [sin|cos](A jj) rotated by
block phases 128*A*(b-m) and summed over blocks b<m (tiny masked matmul).

Sharded over 8 NeuronCores: core m computes output rows [128m, 128m+128).
SPMD: identical program, per-core behavior comes only from input data
(x window slice, block-phase grids, block mask).  Host prep is limited to
layout/replication and affine iota*weight phase grids (pre-wrapped into
[-pi, pi) because the ACT Sin LUT is only accurate there); all sines,
x contractions and T^2-scale work happen on device.

Partition layout: p = c*32 + h (C_in=4 channels x H=32 hidden = 128).
"""

import sys
from pathlib import Path

import numpy as np

for _p in ("/opt/trn_rl_repo",):
    if _p not in sys.path and Path(_p).exists():
        sys.path.insert(0, _p)

import concourse.bass as bass
import concourse.bacc as bacc
import concourse.tile as tile
from concourse import mybir
from concourse.bass_utils import run_bass_kernel_spmd

F32 = mybir.dt.float32
F32R = mybir.dt.float32r
BF16 = mybir.dt.bfloat16
F16 = mybir.dt.float16
PI = float(np.pi)
PI2 = float(np.pi / 2)
T, C, O, H, P, M = 1024, 4, 2, 32, 128, 8

# D2 (2-byte tensor) column offsets.  bf16 columns hold bf16 data; "grid"
# columns hold raw fp16 bits (bitcast to F16 on device before ACT).
D_UT = 0          # [128, 128] bf16 upper-tri (jj <= ii)
D_XWIN = 128      # [128, 4]   bf16 own x window
D_QTSG = 132      # [128, 256] fp16 wrap(theta)        -> sin side
D_QTCG = 388      # [128, 256] fp16 wrap(theta + pi/2) -> cos side
D_ARGL = 644      # [128, 64]  fp16 [A_h*jj | A_h*jj + pi/2] window phases
D_ARGBS = 708     # [32, 32]   fp16 wrap(128*A*(b-m))        (rows 0:32)
D_ARGBC = 740     # [32, 32]   fp16 wrap(128*A*(b-m) + pi/2) (rows 0:32)
D_MASK4 = 772     # [32, 128]  bf16 mask4[(b,c),(c',h)] = (c==c')&(b<m)
D_B2 = 900        # [4, 1]     bf16 b2 (rows 0:4)
D_N = 901

# DW (float32r tensor) columns
W_W2 = 0          # [128, 1] w2 tiled over c
W_E4M = 1         # [32, 4]  E4M[(b,c), c'] = (c==c') & (b<m)  (rows 0:32)
W_XBLK = 5        # [128, 32] x blocked [jj, (b,c)] (plain fp32 bits)
W_N = 37

ACT_F32R = True   # Scalar ACT writes float32r directly (else cast on DVE)

_nc_cache = {}


def _build_nc():
    nc = bacc.Bacc()
    d2 = nc.dram_tensor("d2", [P, D_N], BF16, kind="ExternalInput")
    dw = nc.dram_tensor("dw", [P, W_N], F32R, kind="ExternalInput")
    y = nc.dram_tensor("y", [1, O, P], F32, kind="ExternalOutput")

    Sin = mybir.ActivationFunctionType.Sin
    QT_DT = F32R if ACT_F32R else F32

    with tile.TileContext(nc) as tc:
        with (
            tc.tile_pool(name="sb", bufs=1) as sb,
            tc.tile_pool(name="ps", bufs=1, space="PSUM") as ps,
        ):
            d2_sb = sb.tile([P, D_N], BF16)
            dw_sb = sb.tile([P, W_N], F32R)
            nc.sync.dma_start(out=d2_sb[:], in_=d2[:])
            nc.scalar.dma_start(out=dw_sb[:], in_=dw[:])

            def grid(off, n, rows=P):
                return d2_sb[0:rows, off:off + n].bitcast(F16)

            # dummy sin with no upstream deps: forces the ACT Sin table
            # load to happen at t=0 instead of serializing behind the DMAs.
            # The LUT table is keyed on the in/out dtype pair, so every ACT
            # here reads fp16 and writes an fp32-class dtype (one table).
            pi2h = sb.tile([P, 1], F16)
            nc.vector.memset(pi2h[:], PI2)
            warm = sb.tile([P, 1], F32)
            nc.scalar.activation(warm[:], pi2h[:], Sin)

            # ---- trig tables (one fused ACT per grid pair) ----
            TLp = sb.tile([P, 2 * H + 1], F32)    # [jj, sin|cos|ones]
            nc.scalar.activation(TLp[:, 0:2 * H], grid(D_ARGL, 2 * H), Sin)
            nc.vector.memset(TLp[:, 2 * H:2 * H + 1], 1.0)
            phSC = sb.tile([H, 2 * H], F32)       # block phases [(b,c), h]
            nc.scalar.activation(phSC[:], grid(D_ARGBS, 2 * H, rows=H), Sin)
            phS, phC = phSC[:, 0:H], phSC[:, H:2 * H]
            QT = sb.tile([P, 2, O, P], QT_DT)     # query side [p, s|c, o, ii]
            nc.scalar.activation(
                QT[:].rearrange("p t o i -> p (t o i)"),
                grid(D_QTSG, 2 * O * P), Sin)
            QTs, QTc = QT[:, 0], QT[:, 1]

            # ---- window products R[jj, (c,h)] = trig[jj,h] * xwin[jj,c] ----
            R_s = sb.tile([P, C, H], BF16)
            R_c = sb.tile([P, C, H], BF16)
            tl_s = TLp[:, 0:H].unsqueeze(1).broadcast_to([P, C, H])
            tl_c = TLp[:, H:2 * H].unsqueeze(1).broadcast_to([P, C, H])
            xw_b = d2_sb[:, D_XWIN:D_XWIN + C].unsqueeze(2).broadcast_to([P, C, H])
            nc.vector.tensor_mul(R_s[:], tl_s, xw_b)
            nc.vector.tensor_mul(R_c[:], tl_c, xw_b)

            # ---- PE: history partials, then window prefix sums ----
            p_ps = ps.tile([H, 2 * H + 1], F32)   # P[(b,c), (sin|cos,h)|ones]
            nc.tensor.matmul(p_ps[:], dw_sb[:, W_XBLK:W_XBLK + H].bitcast(F32),
                             TLp[:], start=True, stop=True)
            ut = d2_sb[:, D_UT:D_UT + P]
            pwS = ps.tile([P, P], F32)
            pwC = ps.tile([P, P], F32)
            pwx = ps.tile([C, P], F32)
            nc.tensor.matmul(pwS[:], R_s[:], ut, start=True, stop=True)
            nc.tensor.matmul(pwC[:], R_c[:], ut, start=True, stop=True)
            nc.tensor.matmul(pwx[:], d2_sb[:, D_XWIN:D_XWIN + C], ut,
                             start=True, stop=True)

            # ---- history: rotate partials by block phases, mask+sum b<m ----
            # The masked sum over b lands directly in [(c,h)] partitions:
            # replicate Q over c' with the (c==c')&(b<m) mask on DVE, then
            # contract the 32 (b,c)-partitions against a ones column on PE
            # (out partitions = stationary free dim).  No transpose DMA.
            Ps, Pc = p_ps[:, 0:H], p_ps[:, H:2 * H]
            t_a = sb.tile([H, H], F32)
            t_b = sb.tile([H, H], F32)
            Qs_t = sb.tile([H, H], F32)
            Qc_t = sb.tile([H, H], F32)
            nc.vector.tensor_mul(t_a[:], phC, Ps)
            nc.vector.tensor_mul(t_b[:], phS, Pc)
            nc.vector.tensor_add(Qs_t[:], t_a[:], t_b[:])
            nc.vector.tensor_mul(t_a[:], phC, Pc)
            nc.vector.tensor_mul(t_b[:], phS, Ps)
            nc.vector.tensor_sub(Qc_t[:], t_a[:], t_b[:])
            m4 = d2_sb[0:H, D_MASK4:D_MASK4 + P].rearrange(
                "p (c h) -> p c h", c=C)
            Qs4 = sb.tile([H, C, H], F32)
            Qc4 = sb.tile([H, C, H], F32)
            nc.vector.tensor_mul(Qs4[:], Qs_t[:].unsqueeze(1).broadcast_to([H, C, H]), m4)
            nc.vector.tensor_mul(Qc4[:], Qc_t[:].unsqueeze(1).broadcast_to([H, C, H]), m4)
            ones32 = sb.tile([H, 1], F32)
            nc.vector.memset(ones32[:], 1.0)
            pones = sb.tile([H, 1], F32)
            nc.vector.tensor_copy(pones[:], p_ps[:, 2 * H:2 * H + 1])
            hist2 = ps.tile([P, 2], F32)
            qflat = lambda ap: ap.rearrange("p c h -> p (c h)")
            nc.tensor.matmul(hist2[:, 0:1], qflat(Qs4[:]), ones32[:],
                             start=True, stop=True)
            nc.tensor.matmul(hist2[:, 1:2], qflat(Qc4[:]), ones32[:],
                             start=True, stop=True)
            pcx4 = ps.tile([C, 1], F32)
            nc.tensor.matmul(pcx4[:], dw_sb[0:H, W_E4M:W_E4M + C].bitcast(F32),
                             pones[:], start=True, stop=True)

            # ---- combine on DVE (no col dependency: hist goes via PE) ----
            G1 = sb.tile([P, O, P], F32R)
            G2 = sb.tile([P, O, P], F32R)
            pwS_b = pwS[:].unsqueeze(1).broadcast_to([P, O, P])
            pwC_b = pwC[:].unsqueeze(1).broadcast_to([P, O, P])
            nc.vector.tensor_mul(G1[:], pwS_b, QTc)
            nc.vector.tensor_mul(G2[:], pwC_b, QTs)
            QTs_r, QTc_r = QTs, QTc
            # b2 term: t4x2[c, (o,ii)] = pwx + pcx, replicated over o
            t4x2 = sb.tile([C, O, P], BF16)
            pwx_b = pwx[:].unsqueeze(1).broadcast_to([C, O, P])
            nc.vector.tensor_scalar_add(t4x2[:], pwx_b, pcx4[:])
            # w2-scaled hist columns feed the final contraction directly
            wcol_s = sb.tile([P, 1], F32R)
            wcol_c = sb.tile([P, 1], F32R)
            nc.vector.tensor_mul(wcol_s[:], dw_sb[:, W_W2:W_W2 + 1], hist2[:, 0:1])
            nc.vector.tensor_mul(wcol_c[:], dw_sb[:, W_W2:W_W2 + 1], hist2[:, 1:2])

            # ---- final contraction over p (and c for the b2 term) ----
            yterm = ps.tile([1, O * P], F32)
            w2col = dw_sb[:, W_W2:W_W2 + 1]
            flat = lambda ap: ap.rearrange("p o i -> p (o i)")
            nc.tensor.matmul(yterm[:], w2col, flat(G1[:]), start=True, stop=False)
            nc.tensor.matmul(yterm[:], w2col, flat(G2[:]), start=False, stop=False)
            nc.tensor.matmul(yterm[:], wcol_s[:], flat(QTc_r), start=False, stop=False)
            nc.tensor.matmul(yterm[:], wcol_c[:], flat(QTs_r), start=False, stop=False)
            nc.tensor.matmul(yterm[:], d2_sb[0:C, D_B2:D_B2 + 1],
                             flat(t4x2[:]), start=False, stop=True)
            ysb = sb.tile([1, O * P], F32)
            nc.vector.tensor_copy(ysb[:], yterm[:])
            nc.sync.dma_start(out=y[:].rearrange("p o i -> p (o i)"), in_=ysb[:])
    nc.finalize()
    return nc


def _wrap(v):
    return (v + np.pi) % (2 * np.pi) - np.pi


def _host_inputs(x, w1, b1, w2, b2):
    """Per-core input maps.  Host does layout/replication/masking and
    affine iota*weight phase grids (pre-wrapped for the LUT range)."""
    bf16 = mybir.dt.np(BF16)
    x = np.ascontiguousarray(x, np.float32)
    w1 = np.asarray(w1, np.float64)
    b1 = np.asarray(b1, np.float64)
    w2 = np.asarray(w2, np.float32)
    b2 = np.asarray(b2, np.float32)

    A = w1[:, 0] / T                                    # [H]
    jj = np.arange(P, dtype=np.float64)
    argL = np.outer(jj, A)                              # [128, 32]
    cidx = np.arange(C, dtype=np.float64)
    off0 = cidx[:, None] * w1[None, :, 1] + b1[None, :] # [C, H]
    oidx = np.arange(O, dtype=np.float64)
    theta = (off0[:, :, None, None]
             + oidx[None, None, :, None] * w1[None, :, 2, None, None]
             - A[None, :, None, None] * jj[None, None, None, :])  # [C,H,O,P]
    qtsg = _wrap(theta).reshape(P, O * P)
    qtcg = _wrap(theta + np.pi / 2).reshape(P, O * P)

    ut = np.triu(np.ones((P, P), np.float32))
    xblk = x.reshape(M, P, C).transpose(1, 0, 2).reshape(P, M * C)

    d2_base = np.zeros((P, D_N), dtype=bf16)
    d2_base[:, D_UT:D_UT + P] = ut.astype(bf16)
    d2_base[:, D_QTSG:D_QTSG + O * P] = qtsg.astype(np.float16).view(bf16)
    d2_base[:, D_QTCG:D_QTCG + O * P] = qtcg.astype(np.float16).view(bf16)
    d2_base[:, D_ARGL:D_ARGL + H] = argL.astype(np.float16).view(bf16)
    d2_base[:, D_ARGL + H:D_ARGL + 2 * H] = (argL + np.pi / 2).astype(np.float16).view(bf16)
    d2_base[0:C, D_B2] = np.full(C, b2[0]).astype(bf16)

    dw_base = np.zeros((P, W_N), np.float32)
    dw_base[:, W_W2] = np.tile(w2[0], C)
    dw_base[:, W_XBLK:W_XBLK + H] = xblk

    bvals = np.repeat(np.arange(M, dtype=np.float64), C)  # [(b,c)] -> b
    cvals = np.tile(np.arange(C), M)                      # [(b,c)] -> c
    in_maps = []
    for m in range(M):
        d2m = d2_base.copy()
        d2m[:, D_XWIN:D_XWIN + C] = x[P * m:P * (m + 1)].astype(bf16)
        argB = np.outer(P * (bvals - m), A)               # [32, 32]
        d2m[0:H, D_ARGBS:D_ARGBS + H] = _wrap(argB).astype(np.float16).view(bf16)
        d2m[0:H, D_ARGBC:D_ARGBC + H] = _wrap(argB + np.pi / 2).astype(np.float16).view(bf16)
        dwm = dw_base.copy()
        e4m = (cvals[:, None] == np.arange(C)[None, :]) & (bvals[:, None] < m)
        dwm[0:H, W_E4M:W_E4M + C] = e4m.astype(np.float32)
        mask4 = np.repeat(e4m, H, axis=1)                 # [32, 128]
        d2m[0:H, D_MASK4:D_MASK4 + P] = mask4.astype(bf16)
        in_maps.append({"d2": d2m, "dw": dwm})
    return in_maps


def kernel(x, t, w1, b1, w2, b2, out_channels):
    if "nc" not in _nc_cache:
        _nc_cache["nc"] = _build_nc()
    nc = _nc_cache["nc"]
    in_maps = _host_inputs(x, w1, b1, w2, b2)
    res = run_bass_kernel_spmd(nc, in_maps, core_ids=list(range(M)))
    y = np.empty((T, O), np.float32)
    for m in range(M):
        ym = np.asarray(res.results[m]["y"]).reshape(O, P)
        y[P * m:P * (m + 1), :] = ym.T
    return y
